# revision 1
# baseline (speedup 1.0000x reference)
"""InterpretableMultiHeadAttention kernel for 8 Trainium2 NeuronCores.

Math (per batch b): q/k = x@Wq/k + b; per-head logits = q_h k_h^T/sqrt(dh);
probs = sparsemax(logits); shared V = head-mean of v (linear -> fold into a
(D, dh) weight); out = concat_h(probs_h @ v_shared) @ Wo + bo;
avg_attention = mean_h probs.

Sharding: core c handles batch b=c//2, head-group g=c%2 (8 of 16 heads).
Per-core partial x_out (via Wo row-block) and partial head-sum of probs are
combined on host.

Everything on-device is computed transposed (queries on the free axis) so
every matmul consumes natural layouts; the host transposes the two big
outputs at the end.

Sparsemax per query row: top-8 extraction (nc.vector.max, sorted desc),
closed-form tau* = max_j (cumsum_j - 1)/j over the sorted prefix.  Rows whose
support size is >= 8 are flagged (z_8 > t_8) and corrected exactly on the
host (~1% of rows for this distribution).  tau is broadcast along the free
axis by a per-i-tile PE transpose plus a rank-1 accumulate-matmul, so the
final probs come out of a single fused Relu(PSUM) pass.

Matmuls run as float32r (tf32-class, 4x faster than fp32 on PE).
"""

import sys

sys.path.insert(0, "/opt/trn_rl_repo")

import numpy as np
from contextlib import ExitStack

import concourse.bacc as bacc
import concourse.mybir as mybir
import concourse.tile as tile
from concourse.bass_utils import run_bass_kernel_spmd
from concourse.masks import make_identity

F32 = mybir.dt.float32
F32R = mybir.dt.float32r
AX = mybir.AxisListType
ALU = mybir.AluOpType
ACTF = mybir.ActivationFunctionType

N_CORES = 8
P = 128
B, S, D = 4, 1024, 1024
H = 16                      # total heads
HG = 8                      # heads per core (head-group)
DH = D // H                 # 64
GW = D // 2                 # 512 = per-group projection width
NT = S // P                 # 8 i/j tiles
_cached = {}


def _build():
    nc = bacc.Bacc("TRN2", target_bir_lowering=False, debug=False,
                   num_devices=N_CORES)

    xT_d = nc.dram_tensor("xT", [D, S], F32R, kind="ExternalInput").ap()
    wq_d = nc.dram_tensor("wq", [D, GW], F32R, kind="ExternalInput").ap()
    wk_d = nc.dram_tensor("wk", [D, GW], F32R, kind="ExternalInput").ap()
    wv_d = nc.dram_tensor("wv", [D, DH], F32R, kind="ExternalInput").ap()
    wo_d = nc.dram_tensor("wo", [GW, D], F32R, kind="ExternalInput").ap()
    bq_d = nc.dram_tensor("bq", [1, GW], F32R, kind="ExternalInput").ap()
    bk_d = nc.dram_tensor("bk", [1, GW], F32R, kind="ExternalInput").ap()
    bv_d = nc.dram_tensor("bv", [1, DH], F32R, kind="ExternalInput").ap()
    ones_d = nc.dram_tensor("ones", [1, S], F32R, kind="ExternalInput").ap()

    xoutT_d = nc.dram_tensor("xoutT", [D, S], F32, kind="ExternalOutput").ap()
    avgT_d = nc.dram_tensor("avgT", [S, S], F32, kind="ExternalOutput").ap()
    tau_d = nc.dram_tensor("tau", [P, HG * NT], F32, kind="ExternalOutput").ap()
    flag_d = nc.dram_tensor("flag", [P, HG * NT], F32, kind="ExternalOutput").ap()

    with tile.TileContext(nc) as tc, ExitStack() as es:
        sb = es.enter_context(tc.tile_pool(name="persist", bufs=1))
        psA = es.enter_context(tc.tile_pool(name="psA", bufs=3, space="PSUM"))
        psB = es.enter_context(tc.tile_pool(name="psB", bufs=2, space="PSUM"))
        psO = es.enter_context(tc.tile_pool(name="psO", bufs=2, space="PSUM"))
        psT = es.enter_context(tc.tile_pool(name="psT", bufs=1, space="PSUM"))
        if True:

            # ---- constants ----
            ident = sb.tile([P, P], F32)
            make_identity(nc, ident[:])
            ones_r = sb.tile([1, S], F32R)
            nc.sync.dma_start(out=ones_r[:], in_=ones_d)
            recip = sb.tile([P, 8], F32)
            for j in range(8):
                nc.vector.memset(recip[:, j:j + 1], 1.0 / (j + 1))

            # ---- persistent SBUF tensors ----
            qT = [sb.tile([P, S], F32R, name=f"qT{i}") for i in range(4)]
            kT = [sb.tile([P, S], F32R, name=f"kT{i}") for i in range(4)]
            vsh = [sb.tile([P, DH], F32R, name=f"vsh{i}") for i in range(NT)]
            outT = [sb.tile([P, S], F32R, name=f"outT{i}") for i in range(4)]
            avg = [sb.tile([P, S], F32, name=f"avg{i}") for i in range(NT)]
            wo_sb = [sb.tile([P, S], F32R, name=f"wo{i}") for i in range(4)]
            flags = sb.tile([P, HG * NT], F32)

            for i in range(4):
                nc.sync.dma_start(out=wo_sb[i][:], in_=wo_d[i * P:(i + 1) * P, :])

            # ---- phase 1: q/k/v_shared projections (scoped weights) ----
            with tc.tile_pool(name="ph1", bufs=1) as p1:
                xT_sb = [p1.tile([P, S], F32R, name=f"xT{i}") for i in range(8)]
                wq_sb = [p1.tile([P, GW], F32R, name=f"wq{i}") for i in range(8)]
                wk_sb = [p1.tile([P, GW], F32R, name=f"wk{i}") for i in range(8)]
                wv_sb = [p1.tile([P, DH], F32R, name=f"wv{i}") for i in range(8)]
                bq_sb = p1.tile([1, GW], F32R)
                bk_sb = p1.tile([1, GW], F32R)
                bv_sb = p1.tile([1, DH], F32R)
                for i in range(8):
                    nc.sync.dma_start(out=xT_sb[i][:], in_=xT_d[i * P:(i + 1) * P, :])
                    nc.sync.dma_start(out=wq_sb[i][:], in_=wq_d[i * P:(i + 1) * P, :])
                    nc.sync.dma_start(out=wk_sb[i][:], in_=wk_d[i * P:(i + 1) * P, :])
                    nc.sync.dma_start(out=wv_sb[i][:], in_=wv_d[i * P:(i + 1) * P, :])
                nc.sync.dma_start(out=bq_sb[:], in_=bq_d)
                nc.sync.dma_start(out=bk_sb[:], in_=bk_d)
                nc.sync.dma_start(out=bv_sb[:], in_=bv_d)

                # qT/kT: out[nq 128, s 512] = sum_d w[d, nq] * xT[d, s] (+ bias)
                for w_sb, b_sb, dst in ((wq_sb, bq_sb, qT), (wk_sb, bk_sb, kT)):
                    for m in range(4):          # nq tile
                        for sh in range(2):     # s half
                            ps = psA.tile([P, GW], F32, tag="psA")
                            nc.tensor.matmul(
                                ps[:], lhsT=b_sb[0:1, m * P:(m + 1) * P],
                                rhs=ones_r[0:1, :GW], start=True, stop=False)
                            for kc in range(8):
                                nc.tensor.matmul(
                                    ps[:],
                                    lhsT=w_sb[kc][:, m * P:(m + 1) * P],
                                    rhs=xT_sb[kc][:, sh * GW:(sh + 1) * GW],
                                    start=False, stop=(kc == 7))
                            nc.scalar.copy(
                                out=dst[m][:, sh * GW:(sh + 1) * GW], in_=ps[:])

                # v_shared: out[s 128, nv 64] = sum_d xT[d, s-tile] * wv[d, nv]
                for st in range(NT):
                    ps = psO.tile([P, GW], F32, tag="psO")
                    nc.tensor.matmul(
                        ps[:, :DH], lhsT=ones_r[0:1, :P], rhs=bv_sb[0:1, :],
                        start=True, stop=False)
                    for kc in range(8):
                        nc.tensor.matmul(
                            ps[:, :DH],
                            lhsT=xT_sb[kc][:, st * P:(st + 1) * P],
                            rhs=wv_sb[kc][:], start=False, stop=(kc == 7))
                    nc.scalar.copy(out=vsh[st][:], in_=ps[:, :DH])

            zp = es.enter_context(tc.tile_pool(name="zpool", bufs=3))
            pp = es.enter_context(tc.tile_pool(name="probs", bufs=9))
            sp = es.enter_context(tc.tile_pool(name="small", bufs=4))
            rp = es.enter_context(tc.tile_pool(name="rowp", bufs=2))

            # ---- phase 2: per-head attention ----
            for h in range(HG):
                qt = h // 2           # which qT/kT tile holds this head
                base = (h % 2) * DH   # partition base within the tile (0 or 64)
                negtau = rp.tile([1, S], F32R, tag="negtau")
                tau_h = sp.tile([P, NT], F32, tag="tau_h")

                # --- tau extraction (layout A: queries on partitions) ---
                for it in range(NT):
                    zA = zp.tile([P, S], F32, tag="zA")
                    for jh in range(2):
                        ps = psA.tile([P, GW], F32, tag="psA")
                        nc.tensor.matmul(
                            ps[:],
                            lhsT=qT[qt][base:base + DH, it * P:(it + 1) * P],
                            rhs=kT[qt][base:base + DH, jh * GW:(jh + 1) * GW],
                            start=True, stop=True)
                        nc.scalar.copy(out=zA[:, jh * GW:(jh + 1) * GW], in_=ps[:])
                    top8 = sp.tile([P, 8], F32, tag="top8")
                    nc.vector.max(out=top8[:], in_=zA[:])
                    tj = sp.tile([P, 8], F32, tag="tj")
                    nc.vector.tensor_tensor_scan(
                        out=tj[:], data0=top8[:], data1=top8[:],
                        initial=0.0, op0=ALU.add, op1=ALU.bypass)
                    nc.vector.tensor_scalar_add(tj[:], tj[:], -1.0)
                    nc.vector.tensor_tensor(out=tj[:], in0=tj[:], in1=recip[:],
                                            op=ALU.mult)
                    nc.vector.tensor_reduce(out=tau_h[:, it:it + 1], in_=tj[:],
                                            axis=AX.X, op=ALU.max)
                    nc.vector.tensor_tensor(
                        out=flags[:, h * NT + it:h * NT + it + 1],
                        in0=top8[:, 7:8], in1=tj[:, 7:8], op=ALU.is_gt)
                    # transpose tau column -> (1, 128) row chunk, negated
                    pt = psT.tile([1, P], F32, tag="psT")
                    nc.tensor.transpose(pt[:], tau_h[:, it:it + 1], ident[:])
                    nc.scalar.mul(out=negtau[0:1, it * P:(it + 1) * P],
                                  in_=pt[:], mul=-1.0)

                nc.sync.dma_start(out=tau_d[:, h * NT:(h + 1) * NT], in_=tau_h[:])

                # --- probsT (layout B: keys on partitions) + avg accumulation ---
                probs_h = []
                for jt in range(NT):
                    pr = pp.tile([P, S], F32R, tag="probs")
                    probs_h.append(pr)
                    for ih in range(2):
                        ps = psB.tile([P, GW], F32, tag="psB")
                        nc.tensor.matmul(
                            ps[:],
                            lhsT=kT[qt][base:base + DH, jt * P:(jt + 1) * P],
                            rhs=qT[qt][base:base + DH, ih * GW:(ih + 1) * GW],
                            start=True, stop=False)
                        nc.tensor.matmul(
                            ps[:], lhsT=ones_r[0:1, :P],
                            rhs=negtau[0:1, ih * GW:(ih + 1) * GW],
                            start=False, stop=True, skip_group_check=True)
                        nc.scalar.activation(
                            out=pr[:, ih * GW:(ih + 1) * GW], in_=ps[:],
                            func=ACTF.Relu)
                    if h == 0:
                        nc.vector.tensor_copy(out=avg[jt][:],
                                              in_=probs_h[jt][:].bitcast(F32))
                    else:
                        nc.vector.tensor_tensor(
                            out=avg[jt][:], in0=avg[jt][:],
                            in1=probs_h[jt][:].bitcast(F32), op=ALU.add)

                # --- out_hT[nv, i] = sum_j vsh[j, nv] * probsT[j, i] ---
                for ih in range(2):
                    ps = psO.tile([P, GW], F32, tag="psO")
                    for jt in range(NT):
                        nc.tensor.matmul(
                            ps[:DH, :],
                            lhsT=vsh[jt][:],
                            rhs=probs_h[jt][:, ih * GW:(ih + 1) * GW],
                            start=(jt == 0), stop=(jt == 7))
                    nc.scalar.copy(
                        out=outT[qt][base:base + DH, ih * GW:(ih + 1) * GW],
                        in_=ps[:DH, :])

            # ---- phase 3: x_outT[dcol, i] = sum_nc wo[nc, dcol] outT[nc, i] ----
            for m in range(8):
                for ih in range(2):
                    ps = psB.tile([P, GW], F32, tag="psB")
                    for kc in range(4):
                        nc.tensor.matmul(
                            ps[:],
                            lhsT=wo_sb[kc][:, m * P:(m + 1) * P],
                            rhs=outT[kc][:, ih * GW:(ih + 1) * GW],
                            start=(kc == 0), stop=(kc == 3))
                    xo = zp.tile([P, GW], F32, tag="xo")
                    nc.scalar.copy(out=xo[:], in_=ps[:])
                    nc.sync.dma_start(
                        out=xoutT_d[m * P:(m + 1) * P, ih * GW:(ih + 1) * GW],
                        in_=xo[:])

            for jt in range(NT):
                nc.sync.dma_start(out=avgT_d[jt * P:(jt + 1) * P, :], in_=avg[jt][:])
            nc.sync.dma_start(out=flag_d, in_=flags[:])

    nc.compile()
    return nc


def _sparsemax_row(z):
    zs = -np.sort(-z)
    cs = np.cumsum(zs)
    k = np.arange(1, z.shape[0] + 1)
    supp = (1.0 + k * zs) > cs
    ksz = int(supp.sum())
    tau = (cs[ksz - 1] - 1.0) / ksz
    return np.maximum(z - tau, 0.0)


def kernel(x, Wq, bq, Wk, bk, Wv, bv, Wo, bo):
    x = np.asarray(x, dtype=np.float32)
    Wq = np.asarray(Wq, dtype=np.float32); bq = np.asarray(bq, dtype=np.float32)
    Wk = np.asarray(Wk, dtype=np.float32); bk = np.asarray(bk, dtype=np.float32)
    Wv = np.asarray(Wv, dtype=np.float32); bv = np.asarray(bv, dtype=np.float32)
    Wo = np.asarray(Wo, dtype=np.float32); bo = np.asarray(bo, dtype=np.float32)

    if "nc" not in _cached:
        _cached["nc"] = _build()
    nc = _cached["nc"]

    wv_sh = Wv.reshape(D, H, DH).mean(axis=1)          # (D, 64)
    bv_sh = bv.reshape(H, DH).mean(axis=0)             # (64,)

    in_maps = []
    for c in range(N_CORES):
        b_idx, g = c // 2, c % 2
        cols = slice(g * GW, (g + 1) * GW)
        in_maps.append({
            "xT": np.ascontiguousarray(x[b_idx].T),
            "wq": np.ascontiguousarray(Wq[:, cols] * 0.125),
            "wk": np.ascontiguousarray(Wk[:, cols]),
            "wv": np.ascontiguousarray(wv_sh),
            "wo": np.ascontiguousarray(Wo[cols, :]),
            "bq": (bq[cols] * 0.125).reshape(1, GW).copy(),
            "bk": bk[cols].reshape(1, GW).copy(),
            "bv": bv_sh.reshape(1, DH).copy(),
            "ones": np.ones((1, S), dtype=np.float32),
        })

    res = run_bass_kernel_spmd(nc, in_maps, list(range(N_CORES)))
    r = res.results

    x_out = np.empty((B, S, D), dtype=np.float32)
    avg = np.empty((B, S, S), dtype=np.float32)
    for b_idx in range(B):
        x_out[b_idx] = (r[2 * b_idx]["xoutT"] + r[2 * b_idx + 1]["xoutT"]).T + bo
        avg[b_idx] = (r[2 * b_idx]["avgT"] + r[2 * b_idx + 1]["avgT"]).T / H

    # ---- host fixup of rows with sparsemax support >= 8 ----
    flagged = []   # (b, head, i, tau_dev)
    for c in range(N_CORES):
        fl = r[c]["flag"]
        taus = r[c]["tau"]
        ps, gs = np.nonzero(fl > 0.5)
        for p, g64 in zip(ps, gs):
            head = (c % 2) * HG + g64 // NT
            i = (g64 % NT) * P + int(p)
            flagged.append((c // 2, head, i, float(taus[p, g64])))

    if flagged:
        bs_needed = sorted({f[0] for f in flagged})
        qkv_cache = {}
        for b_idx in bs_needed:
            qkv_cache[b_idx] = (
                x[b_idx] @ Wq + bq,
                x[b_idx] @ Wk + bk,
                x[b_idx] @ wv_sh + bv_sh,
            )
        scale = 1.0 / np.sqrt(DH)
        for b_idx, head, i, tau_dev in flagged:
            qb, kb, vb = qkv_cache[b_idx]
            hc = slice(head * DH, (head + 1) * DH)
            z = (qb[i, hc] @ kb[:, hc].T) * scale          # (S,)
            probs_new = _sparsemax_row(z)
            probs_old = np.maximum(z - tau_dev, 0.0)
            delta = probs_new - probs_old
            avg[b_idx, i, :] += delta / H
            x_out[b_idx, i, :] += (delta @ vb) @ Wo[hc, :]

    return x_out, avg



# revision 2
# speedup vs baseline: 3.0031x; 3.0031x over previous
"""InterpretableMultiHeadAttention kernel for 8 Trainium2 NeuronCores.

Math (per batch b): q/k = x@Wq/k + b; per-head logits = q_h k_h^T/sqrt(dh);
probs = sparsemax(logits); shared V = head-mean of v (linear -> fold into a
(D, dh) weight); out = concat_h(probs_h @ v_shared) @ Wo + bo;
avg_attention = mean_h probs.

Sharding: core c handles batch b=c//2, head-group g=c%2 (8 of 16 heads).

Wire-format optimization (the metric is dominated by host<->device transfer
over the axon tunnel, ~19 ms/MB each way):
  * everything on the wire is fp16 (intermediate math stays f32 in PSUM);
  * inputs are deduplicated: each core uploads only a disjoint shard of
    x^T / Wq / Wk / Wo / v_shared and the full tensors are reassembled
    on-device with AllGather collectives (pair groups for x, quad groups
    for the weights, all-8 for v_shared);
  * outputs are pair-ReduceScattered on-device so each core downloads a
    disjoint half of the final x_out^T and avg^T for its batch (the
    head-group partial sums never cross the tunnel).

Everything on-device is computed transposed (queries on the free axis) so
every matmul consumes natural layouts; the host transposes the two big
outputs at the end.

Sparsemax per query row: top-8 extraction (nc.vector.max, sorted desc),
closed-form tau* = max_j (cumsum_j - 1)/j over the sorted prefix.  Rows whose
support size is >= 8 are flagged (z_8 > t_8) and corrected exactly on the
host (~1% of rows for this distribution).  tau is broadcast along the free
axis by a per-i-tile PE transpose plus a rank-1 accumulate-matmul, so the
final probs come out of a single fused Relu(PSUM) pass.
"""

import sys

sys.path.insert(0, "/opt/trn_rl_repo")

import numpy as np
from contextlib import ExitStack

import concourse.bacc as bacc
import concourse.mybir as mybir
import concourse.tile as tile
from concourse.bass_utils import run_bass_kernel_spmd
from concourse.masks import make_identity

F32 = mybir.dt.float32
F16 = mybir.dt.float16
AX = mybir.AxisListType
ALU = mybir.AluOpType
ACTF = mybir.ActivationFunctionType

N_CORES = 8
P = 128
B, S, D = 4, 1024, 1024
H = 16                      # total heads
HG = 8                      # heads per core (head-group)
DH = D // H                 # 64
GW = D // 2                 # 512 = per-group projection width
NT = S // P                 # 8 i/j tiles

PAIRS = [[0, 1], [2, 3], [4, 5], [6, 7]]     # {batch} x {head-group g}
QUADS = [[0, 2, 4, 6], [1, 3, 5, 7]]         # same g across batches
ALL8 = [list(range(N_CORES))]
_cached = {}


def _build():
    nc = bacc.Bacc("TRN2", target_bir_lowering=False, debug=False,
                   num_devices=N_CORES)

    # Sharded wire inputs (fp16); full tensors reassembled with AllGather.
    xTh_d = nc.dram_tensor("xTh", [GW, S], F16, kind="ExternalInput").ap()
    wq_d = nc.dram_tensor("wq", [D // 4, GW], F16, kind="ExternalInput").ap()
    wk_d = nc.dram_tensor("wk", [D // 4, GW], F16, kind="ExternalInput").ap()
    wo_d = nc.dram_tensor("wo", [GW // 4, D], F16, kind="ExternalInput").ap()
    wv_d = nc.dram_tensor("wv", [D // 8, DH], F16, kind="ExternalInput").ap()
    bq_d = nc.dram_tensor("bq", [1, GW], F16, kind="ExternalInput").ap()
    bk_d = nc.dram_tensor("bk", [1, GW], F16, kind="ExternalInput").ap()
    bv_d = nc.dram_tensor("bv", [1, DH], F16, kind="ExternalInput").ap()

    # Disjoint fp16 output halves (pair-ReduceScattered on device).
    xoutTh_d = nc.dram_tensor("xoutTh", [GW, S], F16, kind="ExternalOutput").ap()
    avgTh_d = nc.dram_tensor("avgTh", [GW, S], F16, kind="ExternalOutput").ap()
    tau_d = nc.dram_tensor("tau", [P, HG * NT], F16, kind="ExternalOutput").ap()
    flag_d = nc.dram_tensor("flag", [P, HG * NT], F16, kind="ExternalOutput").ap()

    with tile.TileContext(nc) as tc, ExitStack() as es:
        dram = es.enter_context(tc.tile_pool(name="dram", bufs=1, space="DRAM"))
        sb = es.enter_context(tc.tile_pool(name="persist", bufs=1))
        psA = es.enter_context(tc.tile_pool(name="psA", bufs=3, space="PSUM"))
        psB = es.enter_context(tc.tile_pool(name="psB", bufs=2, space="PSUM"))
        psO = es.enter_context(tc.tile_pool(name="psO", bufs=2, space="PSUM"))
        psT = es.enter_context(tc.tile_pool(name="psT", bufs=1, space="PSUM"))
        if True:
            # ---- input gather collectives (DRAM bounce buffers) ----
            xg_i = dram.tile([GW, S], F16)
            xg_o = dram.tile([D, S], F16)
            wqg_i = dram.tile([D // 4, GW], F16)
            wqg_o = dram.tile([D, GW], F16)
            wkg_i = dram.tile([D // 4, GW], F16)
            wkg_o = dram.tile([D, GW], F16)
            wog_i = dram.tile([GW // 4, D], F16)
            wog_o = dram.tile([GW, D], F16)
            wvg_i = dram.tile([D // 8, DH], F16)
            wvg_o = dram.tile([D, DH], F16)

            for bounce, ext in ((xg_i, xTh_d), (wqg_i, wq_d), (wkg_i, wk_d),
                                (wog_i, wo_d), (wvg_i, wv_d)):
                nc.gpsimd.dma_start(bounce[:], ext)
            for groups, b_in, b_out in (
                    (PAIRS, xg_i, xg_o), (QUADS, wqg_i, wqg_o),
                    (QUADS, wkg_i, wkg_o), (QUADS, wog_i, wog_o),
                    (ALL8, wvg_i, wvg_o)):
                nc.gpsimd.collective_compute(
                    "AllGather", ALU.bypass, replica_groups=groups,
                    ins=[b_in.opt()], outs=[b_out.opt()])

            # ---- constants ----
            ident = sb.tile([P, P], F32)
            make_identity(nc, ident[:])
            ones_r = sb.tile([1, S], F16)
            nc.vector.memset(ones_r[:], 1.0)
            recip = sb.tile([P, 8], F32)
            for j in range(8):
                nc.vector.memset(recip[:, j:j + 1], 1.0 / (j + 1))

            # ---- persistent SBUF tensors ----
            qT = [sb.tile([P, S], F16, name=f"qT{i}") for i in range(4)]
            kT = [sb.tile([P, S], F16, name=f"kT{i}") for i in range(4)]
            vsh = [sb.tile([P, DH], F16, name=f"vsh{i}") for i in range(NT)]
            outT = [sb.tile([P, S], F16, name=f"outT{i}") for i in range(4)]
            avg = [sb.tile([P, S], F32, name=f"avg{i}") for i in range(NT)]
            wo_sb = [sb.tile([P, S], F16, name=f"wo{i}") for i in range(4)]
            flags = sb.tile([P, HG * NT], F16)
            tau16 = sb.tile([P, HG * NT], F16)

            for i in range(4):
                nc.sync.dma_start(out=wo_sb[i][:], in_=wog_o[i * P:(i + 1) * P, :])

            # ---- phase 1: q/k/v_shared projections (scoped weights) ----
            with tc.tile_pool(name="ph1", bufs=1) as p1:
                xT_sb = [p1.tile([P, S], F16, name=f"xT{i}") for i in range(8)]
                wq_sb = [p1.tile([P, GW], F16, name=f"wq{i}") for i in range(8)]
                wk_sb = [p1.tile([P, GW], F16, name=f"wk{i}") for i in range(8)]
                wv_sb = [p1.tile([P, DH], F16, name=f"wv{i}") for i in range(8)]
                bq_sb = p1.tile([1, GW], F16)
                bk_sb = p1.tile([1, GW], F16)
                bv_sb = p1.tile([1, DH], F16)
                for i in range(8):
                    nc.sync.dma_start(out=xT_sb[i][:], in_=xg_o[i * P:(i + 1) * P, :])
                    nc.sync.dma_start(out=wq_sb[i][:], in_=wqg_o[i * P:(i + 1) * P, :])
                    nc.sync.dma_start(out=wk_sb[i][:], in_=wkg_o[i * P:(i + 1) * P, :])
                    nc.sync.dma_start(out=wv_sb[i][:], in_=wvg_o[i * P:(i + 1) * P, :])
                nc.sync.dma_start(out=bq_sb[:], in_=bq_d)
                nc.sync.dma_start(out=bk_sb[:], in_=bk_d)
                nc.sync.dma_start(out=bv_sb[:], in_=bv_d)

                # qT/kT: out[nq 128, s 512] = sum_d w[d, nq] * xT[d, s] (+ bias)
                for w_sb, b_sb, dst in ((wq_sb, bq_sb, qT), (wk_sb, bk_sb, kT)):
                    for m in range(4):          # nq tile
                        for sh in range(2):     # s half
                            ps = psA.tile([P, GW], F32, tag="psA")
                            nc.tensor.matmul(
                                ps[:], lhsT=b_sb[0:1, m * P:(m + 1) * P],
                                rhs=ones_r[0:1, :GW], start=True, stop=False)
                            for kc in range(8):
                                nc.tensor.matmul(
                                    ps[:],
                                    lhsT=w_sb[kc][:, m * P:(m + 1) * P],
                                    rhs=xT_sb[kc][:, sh * GW:(sh + 1) * GW],
                                    start=False, stop=(kc == 7))
                            nc.scalar.copy(
                                out=dst[m][:, sh * GW:(sh + 1) * GW], in_=ps[:])

                # v_shared: out[s 128, nv 64] = sum_d xT[d, s-tile] * wv[d, nv]
                for st in range(NT):
                    ps = psO.tile([P, GW], F32, tag="psO")
                    nc.tensor.matmul(
                        ps[:, :DH], lhsT=ones_r[0:1, :P], rhs=bv_sb[0:1, :],
                        start=True, stop=False)
                    for kc in range(8):
                        nc.tensor.matmul(
                            ps[:, :DH],
                            lhsT=xT_sb[kc][:, st * P:(st + 1) * P],
                            rhs=wv_sb[kc][:], start=False, stop=(kc == 7))
                    nc.scalar.copy(out=vsh[st][:], in_=ps[:, :DH])

            zp = es.enter_context(tc.tile_pool(name="zpool", bufs=3))
            pp = es.enter_context(tc.tile_pool(name="probs", bufs=9))
            sp = es.enter_context(tc.tile_pool(name="small", bufs=4))
            rp = es.enter_context(tc.tile_pool(name="rowp", bufs=2))

            # ---- phase 2: per-head attention ----
            for h in range(HG):
                qt = h // 2           # which qT/kT tile holds this head
                base = (h % 2) * DH   # partition base within the tile (0 or 64)
                negtau = rp.tile([1, S], F16, tag="negtau")
                tau_h = sp.tile([P, NT], F32, tag="tau_h")

                # --- tau extraction (layout A: queries on partitions) ---
                for it in range(NT):
                    zA = zp.tile([P, S], F32, tag="zA")
                    for jh in range(2):
                        ps = psA.tile([P, GW], F32, tag="psA")
                        nc.tensor.matmul(
                            ps[:],
                            lhsT=qT[qt][base:base + DH, it * P:(it + 1) * P],
                            rhs=kT[qt][base:base + DH, jh * GW:(jh + 1) * GW],
                            start=True, stop=True)
                        nc.scalar.copy(out=zA[:, jh * GW:(jh + 1) * GW], in_=ps[:])
                    top8 = sp.tile([P, 8], F32, tag="top8")
                    nc.vector.max(out=top8[:], in_=zA[:])
                    tj = sp.tile([P, 8], F32, tag="tj")
                    nc.vector.tensor_tensor_scan(
                        out=tj[:], data0=top8[:], data1=top8[:],
                        initial=0.0, op0=ALU.add, op1=ALU.bypass)
                    nc.vector.tensor_scalar_add(tj[:], tj[:], -1.0)
                    nc.vector.tensor_tensor(out=tj[:], in0=tj[:], in1=recip[:],
                                            op=ALU.mult)
                    nc.vector.tensor_reduce(out=tau_h[:, it:it + 1], in_=tj[:],
                                            axis=AX.X, op=ALU.max)
                    nc.vector.tensor_tensor(
                        out=flags[:, h * NT + it:h * NT + it + 1],
                        in0=top8[:, 7:8], in1=tj[:, 7:8], op=ALU.is_gt)
                    # transpose tau column -> (1, 128) row chunk, negated
                    pt = psT.tile([1, P], F32, tag="psT")
                    nc.tensor.transpose(pt[:], tau_h[:, it:it + 1], ident[:])
                    nc.scalar.mul(out=negtau[0:1, it * P:(it + 1) * P],
                                  in_=pt[:], mul=-1.0)

                nc.scalar.copy(out=tau16[:, h * NT:(h + 1) * NT], in_=tau_h[:])

                # --- probsT (layout B: keys on partitions) + avg accumulation ---
                probs_h = []
                for jt in range(NT):
                    pr = pp.tile([P, S], F16, tag="probs")
                    probs_h.append(pr)
                    for ih in range(2):
                        ps = psB.tile([P, GW], F32, tag="psB")
                        nc.tensor.matmul(
                            ps[:],
                            lhsT=kT[qt][base:base + DH, jt * P:(jt + 1) * P],
                            rhs=qT[qt][base:base + DH, ih * GW:(ih + 1) * GW],
                            start=True, stop=False)
                        nc.tensor.matmul(
                            ps[:], lhsT=ones_r[0:1, :P],
                            rhs=negtau[0:1, ih * GW:(ih + 1) * GW],
                            start=False, stop=True, skip_group_check=True)
                        nc.scalar.activation(
                            out=pr[:, ih * GW:(ih + 1) * GW], in_=ps[:],
                            func=ACTF.Relu)
                    if h == 0:
                        nc.vector.tensor_copy(out=avg[jt][:], in_=probs_h[jt][:])
                    else:
                        nc.vector.tensor_tensor(
                            out=avg[jt][:], in0=avg[jt][:],
                            in1=probs_h[jt][:], op=ALU.add)

                # --- out_hT[nv, i] = sum_j vsh[j, nv] * probsT[j, i] ---
                for ih in range(2):
                    ps = psO.tile([P, GW], F32, tag="psO")
                    for jt in range(NT):
                        nc.tensor.matmul(
                            ps[:DH, :],
                            lhsT=vsh[jt][:],
                            rhs=probs_h[jt][:, ih * GW:(ih + 1) * GW],
                            start=(jt == 0), stop=(jt == 7))
                    nc.scalar.copy(
                        out=outT[qt][base:base + DH, ih * GW:(ih + 1) * GW],
                        in_=ps[:DH, :])

            # ---- output pair-reduce: avg (scaled by 1/H) and x_outT ----
            av_i = dram.tile([S, S], F16)
            av_o = dram.tile([GW, S], F16)
            xo_i = dram.tile([D, S], F16)
            xo_o = dram.tile([GW, S], F16)

            for jt in range(NT):
                a16 = zp.tile([P, S], F16, tag="a16")
                nc.scalar.mul(out=a16[:], in_=avg[jt][:], mul=1.0 / H)
                nc.sync.dma_start(out=av_i[jt * P:(jt + 1) * P, :], in_=a16[:])

            # ---- phase 3: x_outT[dcol, i] = sum_nc wo[nc, dcol] outT[nc, i] ----
            for m in range(8):
                for ih in range(2):
                    ps = psB.tile([P, GW], F32, tag="psB")
                    for kc in range(4):
                        nc.tensor.matmul(
                            ps[:],
                            lhsT=wo_sb[kc][:, m * P:(m + 1) * P],
                            rhs=outT[kc][:, ih * GW:(ih + 1) * GW],
                            start=(kc == 0), stop=(kc == 3))
                    xo = zp.tile([P, GW], F16, tag="xo")
                    nc.scalar.copy(out=xo[:], in_=ps[:])
                    nc.sync.dma_start(
                        out=xo_i[m * P:(m + 1) * P, ih * GW:(ih + 1) * GW],
                        in_=xo[:])

            for b_in, b_out, ext in ((av_i, av_o, avgTh_d), (xo_i, xo_o, xoutTh_d)):
                nc.gpsimd.collective_compute(
                    "ReduceScatter", ALU.add, replica_groups=PAIRS,
                    ins=[b_in.opt()], outs=[b_out.opt()])
                nc.gpsimd.dma_start(ext, b_out[:])

            nc.sync.dma_start(out=tau_d, in_=tau16[:])
            nc.sync.dma_start(out=flag_d, in_=flags[:])

    nc.compile()
    return nc


def _sparsemax_row(z):
    zs = -np.sort(-z)
    cs = np.cumsum(zs)
    k = np.arange(1, z.shape[0] + 1)
    supp = (1.0 + k * zs) > cs
    ksz = int(supp.sum())
    tau = (cs[ksz - 1] - 1.0) / ksz
    return np.maximum(z - tau, 0.0)


def _make_in_maps(x, Wq, bq, Wk, bk, Wv, bv, Wo, bo):
    wv_sh = Wv.reshape(D, H, DH).mean(axis=1)          # (D, 64)
    bv_sh = bv.reshape(H, DH).mean(axis=0)             # (64,)
    in_maps = []
    for c in range(N_CORES):
        b_idx, g = c // 2, c % 2
        cols = slice(g * GW, (g + 1) * GW)
        q4 = slice(b_idx * (D // 4), (b_idx + 1) * (D // 4))
        in_maps.append({
            "xTh": x[b_idx][:, g * GW:(g + 1) * GW].T.astype(np.float16),
            "wq": (Wq[q4, cols] * 0.125).astype(np.float16),
            "wk": Wk[q4, cols].astype(np.float16),
            "wo": Wo[g * GW + b_idx * P:g * GW + (b_idx + 1) * P, :].astype(np.float16),
            "wv": wv_sh[c * P:(c + 1) * P, :].astype(np.float16),
            "bq": (bq[cols] * 0.125).reshape(1, GW).astype(np.float16),
            "bk": bk[cols].reshape(1, GW).astype(np.float16),
            "bv": bv_sh.reshape(1, DH).astype(np.float16),
        })
    return in_maps, wv_sh, bv_sh


def kernel(x, Wq, bq, Wk, bk, Wv, bv, Wo, bo):
    x = np.asarray(x, dtype=np.float32)
    Wq = np.asarray(Wq, dtype=np.float32); bq = np.asarray(bq, dtype=np.float32)
    Wk = np.asarray(Wk, dtype=np.float32); bk = np.asarray(bk, dtype=np.float32)
    Wv = np.asarray(Wv, dtype=np.float32); bv = np.asarray(bv, dtype=np.float32)
    Wo = np.asarray(Wo, dtype=np.float32); bo = np.asarray(bo, dtype=np.float32)

    if "nc" not in _cached:
        _cached["nc"] = _build()
    nc = _cached["nc"]

    in_maps, wv_sh, bv_sh = _make_in_maps(x, Wq, bq, Wk, bk, Wv, bv, Wo, bo)
    res = run_bass_kernel_spmd(nc, in_maps, list(range(N_CORES)))
    r = res.results

    x_out = np.empty((B, S, D), dtype=np.float32)
    avg = np.empty((B, S, S), dtype=np.float32)
    for b_idx in range(B):
        xoT = np.concatenate(
            [r[2 * b_idx]["xoutTh"], r[2 * b_idx + 1]["xoutTh"]], axis=0)
        x_out[b_idx] = xoT.T.astype(np.float32) + bo
        avT = np.concatenate(
            [r[2 * b_idx]["avgTh"], r[2 * b_idx + 1]["avgTh"]], axis=0)
        avg[b_idx] = avT.T.astype(np.float32)

    # ---- host fixup of rows with sparsemax support >= 8 ----
    flagged = []   # (b, head, i, tau_dev)
    for c in range(N_CORES):
        fl = r[c]["flag"]
        taus = r[c]["tau"]
        ps, gs = np.nonzero(fl > 0.5)
        for p, g64 in zip(ps, gs):
            head = (c % 2) * HG + g64 // NT
            i = (g64 % NT) * P + int(p)
            flagged.append((c // 2, head, i, float(taus[p, g64])))

    if flagged:
        bs_needed = sorted({f[0] for f in flagged})
        qkv_cache = {}
        for b_idx in bs_needed:
            qkv_cache[b_idx] = (
                x[b_idx] @ Wq + bq,
                x[b_idx] @ Wk + bk,
                x[b_idx] @ wv_sh + bv_sh,
            )
        scale = 1.0 / np.sqrt(DH)
        for b_idx, head, i, tau_dev in flagged:
            qb, kb, vb = qkv_cache[b_idx]
            hc = slice(head * DH, (head + 1) * DH)
            z = (qb[i, hc] @ kb[:, hc].T) * scale          # (S,)
            probs_new = _sparsemax_row(z)
            probs_old = np.maximum(z - tau_dev, 0.0)
            delta = probs_new - probs_old
            avg[b_idx, i, :] += delta / H
            x_out[b_idx, i, :] += (delta @ vb) @ Wo[hc, :]

    return x_out, avg


# revision 3
# speedup vs baseline: 4.1475x; 1.3811x over previous
"""InterpretableMultiHeadAttention kernel for 8 Trainium2 NeuronCores.

Math (per batch b): q/k = x@Wq/k + b; per-head logits = q_h k_h^T/sqrt(dh);
probs = sparsemax(logits); shared V = head-mean of v (linear -> fold into a
(D, dh) weight); out = concat_h(probs_h @ v_shared) @ Wo + bo;
avg_attention = mean_h probs.

Sharding: core c handles batch b=c//2, head-group g=c%2 (8 of 16 heads).

Wire-format optimization (the metric is dominated by host<->device transfer
over the axon tunnel, ~19 ms/MB each way plus ~10 ms per array):
  * everything on the wire is fp16 (intermediate math stays f32 in PSUM);
  * each core uploads ONE packed blob holding only a disjoint shard of
    x^T / Wq / Wk / Wo / v_shared; full tensors are reassembled on-device
    with two AllGather collectives (pair groups for x, quad groups for the
    weights);
  * the two big outputs are combined by a single pair-ReduceScatter whose
    rank split hands core (b,0) the summed x_out^T[b] and core (b,1) the
    summed avg^T[b], so each core downloads ONE packed blob and the
    head-group partial sums never cross the tunnel.

Everything on-device is computed transposed (queries on the free axis) so
every matmul consumes natural layouts; the host transposes the two big
outputs at the end.

Sparsemax per query row: top-8 extraction (nc.vector.max, sorted desc),
closed-form tau* = max_j (cumsum_j - 1)/j over the sorted prefix.  Rows whose
support size is >= 8 are flagged (z_8 > t_8) and corrected exactly on the
host (~1% of rows for this distribution).  tau is broadcast along the free
axis by a per-i-tile PE transpose plus a rank-1 accumulate-matmul, so the
final probs come out of a single fused Relu(PSUM) pass.
"""

import sys

sys.path.insert(0, "/opt/trn_rl_repo")

import numpy as np
from contextlib import ExitStack

import concourse.bacc as bacc
import concourse.mybir as mybir
import concourse.tile as tile
from concourse.bass_utils import run_bass_kernel_spmd
from concourse.masks import make_identity

F32 = mybir.dt.float32
F16 = mybir.dt.float16
AX = mybir.AxisListType
ALU = mybir.AluOpType
ACTF = mybir.ActivationFunctionType

N_CORES = 8
P = 128
B, S, D = 4, 1024, 1024
H = 16                      # total heads
HG = 8                      # heads per core (head-group)
DH = D // H                 # 64
GW = D // 2                 # 512 = per-group projection width
NT = S // P                 # 8 i/j tiles

# input blob row offsets (width 1024, fp16)
RX = 0          # xTh               [512, 1024]
RWQ = 512       # wq   [256,512] -> [128, 1024]
RWK = 640       # wk   [256,512] -> [128, 1024]
RWO = 768       # wo               [128, 1024]
RWV = 896       # wv   [256, 64] -> [ 16, 1024]
RB = 912        # row: [bq*0.125 | bk]
RBV = 913       # row: [bv_sh pad]
IN_ROWS = 914
WG_ROWS = RB - RWQ          # 400 rows gathered per quad rank

# output blob row offsets (width 1024, fp16)
OTAU = 1024     # tau  [128, 64] -> [8, 1024]
OFLAG = 1032    # flag [128, 64] -> [8, 1024]
OUT_ROWS = 1040

PAIRS = [[0, 1], [2, 3], [4, 5], [6, 7]]     # {batch} x {head-group g}
QUADS = [[0, 2, 4, 6], [1, 3, 5, 7]]         # same g across batches
_cached = {}


def _build():
    nc = bacc.Bacc("TRN2", target_bir_lowering=False, debug=False,
                   num_devices=N_CORES)

    inb_d = nc.dram_tensor("inb", [IN_ROWS, S], F16, kind="ExternalInput").ap()
    outb_d = nc.dram_tensor("outb", [OUT_ROWS, S], F16, kind="ExternalOutput").ap()

    with tile.TileContext(nc) as tc, ExitStack() as es:
        dram = es.enter_context(tc.tile_pool(name="dram", bufs=1, space="DRAM"))
        sb = es.enter_context(tc.tile_pool(name="persist", bufs=1))
        psA = es.enter_context(tc.tile_pool(name="psA", bufs=3, space="PSUM"))
        psB = es.enter_context(tc.tile_pool(name="psB", bufs=2, space="PSUM"))
        psO = es.enter_context(tc.tile_pool(name="psO", bufs=2, space="PSUM"))
        psT = es.enter_context(tc.tile_pool(name="psT", bufs=1, space="PSUM"))
        if True:
            # ---- input gather collectives (DRAM bounce buffers) ----
            xg_i = dram.tile([GW, S], F16)
            xg_o = dram.tile([D, S], F16)          # full xT
            wg_i = dram.tile([WG_ROWS, S], F16)
            wg_o = dram.tile([4 * WG_ROWS, S], F16)  # 4 rank blocks of weights

            nc.gpsimd.dma_start(xg_i[:], inb_d[RX:RX + GW, :])
            nc.gpsimd.dma_start(wg_i[:], inb_d[RWQ:RB, :])
            nc.gpsimd.collective_compute(
                "AllGather", ALU.bypass, replica_groups=PAIRS,
                ins=[xg_i.opt()], outs=[xg_o.opt()])
            nc.gpsimd.collective_compute(
                "AllGather", ALU.bypass, replica_groups=QUADS,
                ins=[wg_i.opt()], outs=[wg_o.opt()])

            # ---- constants ----
            ident = sb.tile([P, P], F32)
            make_identity(nc, ident[:])
            ones_r = sb.tile([1, S], F16)
            nc.vector.memset(ones_r[:], 1.0)
            recip = sb.tile([P, 8], F32)
            for j in range(8):
                nc.vector.memset(recip[:, j:j + 1], 1.0 / (j + 1))

            # ---- persistent SBUF tensors ----
            qT = [sb.tile([P, S], F16, name=f"qT{i}") for i in range(4)]
            kT = [sb.tile([P, S], F16, name=f"kT{i}") for i in range(4)]
            vsh = [sb.tile([P, DH], F16, name=f"vsh{i}") for i in range(NT)]
            outT = [sb.tile([P, S], F16, name=f"outT{i}") for i in range(4)]
            avg = [sb.tile([P, S], F32, name=f"avg{i}") for i in range(NT)]
            wo_sb = [sb.tile([P, S], F16, name=f"wo{i}") for i in range(4)]
            flags = sb.tile([P, HG * NT], F16)
            tau16 = sb.tile([P, HG * NT], F16)

            for i in range(4):
                # wo block for quad-rank i sits at rows 400*i + 256
                nc.sync.dma_start(out=wo_sb[i][:],
                                  in_=wg_o[WG_ROWS * i + 256:WG_ROWS * i + 384, :])

            # ---- phase 1: q/k/v_shared projections (scoped weights) ----
            with tc.tile_pool(name="ph1", bufs=1) as p1:
                xT_sb = [p1.tile([P, S], F16, name=f"xT{i}") for i in range(8)]
                wq_sb = [p1.tile([P, GW], F16, name=f"wq{i}") for i in range(8)]
                wk_sb = [p1.tile([P, GW], F16, name=f"wk{i}") for i in range(8)]
                wv_sb = [p1.tile([P, DH], F16, name=f"wv{i}") for i in range(8)]
                bq_sb = p1.tile([1, GW], F16)
                bk_sb = p1.tile([1, GW], F16)
                bv_sb = p1.tile([1, DH], F16)
                for i in range(8):
                    rk = WG_ROWS * (i // 2)     # quad-rank block base row
                    half = (i % 2) * 64
                    nc.sync.dma_start(out=xT_sb[i][:], in_=xg_o[i * P:(i + 1) * P, :])
                    # [64, 1024] packed rows -> [128, 512] SBUF tile (same linear order)
                    nc.sync.dma_start(out=wq_sb[i][:],
                                      in_=wg_o[rk + half:rk + half + 64, :])
                    nc.sync.dma_start(out=wk_sb[i][:],
                                      in_=wg_o[rk + 128 + half:rk + 128 + half + 64, :])
                    # [8, 1024] packed rows -> [128, 64] SBUF tile
                    h8 = (i % 2) * 8
                    nc.sync.dma_start(out=wv_sb[i][:],
                                      in_=wg_o[rk + 384 + h8:rk + 384 + h8 + 8, :])
                nc.sync.dma_start(out=bq_sb[:], in_=inb_d[RB:RB + 1, 0:GW])
                nc.sync.dma_start(out=bk_sb[:], in_=inb_d[RB:RB + 1, GW:S])
                nc.sync.dma_start(out=bv_sb[:], in_=inb_d[RBV:RBV + 1, 0:DH])

                # qT/kT: out[nq 128, s 512] = sum_d w[d, nq] * xT[d, s] (+ bias)
                for w_sb, b_sb, dst in ((wq_sb, bq_sb, qT), (wk_sb, bk_sb, kT)):
                    for m in range(4):          # nq tile
                        for sh in range(2):     # s half
                            ps = psA.tile([P, GW], F32, tag="psA")
                            nc.tensor.matmul(
                                ps[:], lhsT=b_sb[0:1, m * P:(m + 1) * P],
                                rhs=ones_r[0:1, :GW], start=True, stop=False)
                            for kc in range(8):
                                nc.tensor.matmul(
                                    ps[:],
                                    lhsT=w_sb[kc][:, m * P:(m + 1) * P],
                                    rhs=xT_sb[kc][:, sh * GW:(sh + 1) * GW],
                                    start=False, stop=(kc == 7))
                            nc.scalar.copy(
                                out=dst[m][:, sh * GW:(sh + 1) * GW], in_=ps[:])

                # v_shared: out[s 128, nv 64] = sum_d xT[d, s-tile] * wv[d, nv]
                for st in range(NT):
                    ps = psO.tile([P, GW], F32, tag="psO")
                    nc.tensor.matmul(
                        ps[:, :DH], lhsT=ones_r[0:1, :P], rhs=bv_sb[0:1, :],
                        start=True, stop=False)
                    for kc in range(8):
                        nc.tensor.matmul(
                            ps[:, :DH],
                            lhsT=xT_sb[kc][:, st * P:(st + 1) * P],
                            rhs=wv_sb[kc][:], start=False, stop=(kc == 7))
                    nc.scalar.copy(out=vsh[st][:], in_=ps[:, :DH])

            zp = es.enter_context(tc.tile_pool(name="zpool", bufs=3))
            pp = es.enter_context(tc.tile_pool(name="probs", bufs=9))
            sp = es.enter_context(tc.tile_pool(name="small", bufs=4))
            rp = es.enter_context(tc.tile_pool(name="rowp", bufs=2))

            # ---- phase 2: per-head attention ----
            for h in range(HG):
                qt = h // 2           # which qT/kT tile holds this head
                base = (h % 2) * DH   # partition base within the tile (0 or 64)
                negtau = rp.tile([1, S], F16, tag="negtau")
                tau_h = sp.tile([P, NT], F32, tag="tau_h")

                # --- tau extraction (layout A: queries on partitions) ---
                for it in range(NT):
                    zA = zp.tile([P, S], F32, tag="zA")
                    for jh in range(2):
                        ps = psA.tile([P, GW], F32, tag="psA")
                        nc.tensor.matmul(
                            ps[:],
                            lhsT=qT[qt][base:base + DH, it * P:(it + 1) * P],
                            rhs=kT[qt][base:base + DH, jh * GW:(jh + 1) * GW],
                            start=True, stop=True)
                        nc.scalar.copy(out=zA[:, jh * GW:(jh + 1) * GW], in_=ps[:])
                    top8 = sp.tile([P, 8], F32, tag="top8")
                    nc.vector.max(out=top8[:], in_=zA[:])
                    tj = sp.tile([P, 8], F32, tag="tj")
                    nc.vector.tensor_tensor_scan(
                        out=tj[:], data0=top8[:], data1=top8[:],
                        initial=0.0, op0=ALU.add, op1=ALU.bypass)
                    nc.vector.tensor_scalar_add(tj[:], tj[:], -1.0)
                    nc.vector.tensor_tensor(out=tj[:], in0=tj[:], in1=recip[:],
                                            op=ALU.mult)
                    nc.vector.tensor_reduce(out=tau_h[:, it:it + 1], in_=tj[:],
                                            axis=AX.X, op=ALU.max)
                    nc.vector.tensor_tensor(
                        out=flags[:, h * NT + it:h * NT + it + 1],
                        in0=top8[:, 7:8], in1=tj[:, 7:8], op=ALU.is_gt)
                    # transpose tau column -> (1, 128) row chunk, negated
                    pt = psT.tile([1, P], F32, tag="psT")
                    nc.tensor.transpose(pt[:], tau_h[:, it:it + 1], ident[:])
                    nc.scalar.mul(out=negtau[0:1, it * P:(it + 1) * P],
                                  in_=pt[:], mul=-1.0)

                nc.scalar.copy(out=tau16[:, h * NT:(h + 1) * NT], in_=tau_h[:])

                # --- probsT (layout B: keys on partitions) + avg accumulation ---
                probs_h = []
                for jt in range(NT):
                    pr = pp.tile([P, S], F16, tag="probs")
                    probs_h.append(pr)
                    for ih in range(2):
                        ps = psB.tile([P, GW], F32, tag="psB")
                        nc.tensor.matmul(
                            ps[:],
                            lhsT=kT[qt][base:base + DH, jt * P:(jt + 1) * P],
                            rhs=qT[qt][base:base + DH, ih * GW:(ih + 1) * GW],
                            start=True, stop=False)
                        nc.tensor.matmul(
                            ps[:], lhsT=ones_r[0:1, :P],
                            rhs=negtau[0:1, ih * GW:(ih + 1) * GW],
                            start=False, stop=True, skip_group_check=True)
                        nc.scalar.activation(
                            out=pr[:, ih * GW:(ih + 1) * GW], in_=ps[:],
                            func=ACTF.Relu)
                    if h == 0:
                        nc.vector.tensor_copy(out=avg[jt][:], in_=probs_h[jt][:])
                    else:
                        nc.vector.tensor_tensor(
                            out=avg[jt][:], in0=avg[jt][:],
                            in1=probs_h[jt][:], op=ALU.add)

                # --- out_hT[nv, i] = sum_j vsh[j, nv] * probsT[j, i] ---
                for ih in range(2):
                    ps = psO.tile([P, GW], F32, tag="psO")
                    for jt in range(NT):
                        nc.tensor.matmul(
                            ps[:DH, :],
                            lhsT=vsh[jt][:],
                            rhs=probs_h[jt][:, ih * GW:(ih + 1) * GW],
                            start=(jt == 0), stop=(jt == 7))
                    nc.scalar.copy(
                        out=outT[qt][base:base + DH, ih * GW:(ih + 1) * GW],
                        in_=ps[:DH, :])

            # ---- output pair-reduce staging: rows 0:1024 x_outT, 1024:2048 avg/H
            rs_i = dram.tile([2 * S, S], F16)
            rs_o = dram.tile([S, S], F16)

            for jt in range(NT):
                a16 = zp.tile([P, S], F16, tag="a16")
                nc.scalar.mul(out=a16[:], in_=avg[jt][:], mul=1.0 / H)
                nc.sync.dma_start(out=rs_i[S + jt * P:S + (jt + 1) * P, :], in_=a16[:])

            # ---- phase 3: x_outT[dcol, i] = sum_nc wo[nc, dcol] outT[nc, i] ----
            for m in range(8):
                for ih in range(2):
                    ps = psB.tile([P, GW], F32, tag="psB")
                    for kc in range(4):
                        nc.tensor.matmul(
                            ps[:],
                            lhsT=wo_sb[kc][:, m * P:(m + 1) * P],
                            rhs=outT[kc][:, ih * GW:(ih + 1) * GW],
                            start=(kc == 0), stop=(kc == 3))
                    xo = zp.tile([P, GW], F16, tag="xo")
                    nc.scalar.copy(out=xo[:], in_=ps[:])
                    nc.sync.dma_start(
                        out=rs_i[m * P:(m + 1) * P, ih * GW:(ih + 1) * GW],
                        in_=xo[:])

            # rank g=0 receives sum(x_outT), rank g=1 receives sum(avgT)/H
            nc.gpsimd.collective_compute(
                "ReduceScatter", ALU.add, replica_groups=PAIRS,
                ins=[rs_i.opt()], outs=[rs_o.opt()])
            nc.gpsimd.dma_start(outb_d[0:S, :], rs_o[:])

            nc.sync.dma_start(out=outb_d[OTAU:OTAU + 8, :], in_=tau16[:])
            nc.sync.dma_start(out=outb_d[OFLAG:OFLAG + 8, :], in_=flags[:])

    nc.compile()
    return nc


def _sparsemax_row(z):
    zs = -np.sort(-z)
    cs = np.cumsum(zs)
    k = np.arange(1, z.shape[0] + 1)
    supp = (1.0 + k * zs) > cs
    ksz = int(supp.sum())
    tau = (cs[ksz - 1] - 1.0) / ksz
    return np.maximum(z - tau, 0.0)


def _make_in_maps(x, Wq, bq, Wk, bk, Wv, bv, Wo, bo):
    wv_sh = Wv.reshape(D, H, DH).mean(axis=1)          # (D, 64)
    bv_sh = bv.reshape(H, DH).mean(axis=0)             # (64,)
    in_maps = []
    for c in range(N_CORES):
        b_idx, g = c // 2, c % 2
        cols = slice(g * GW, (g + 1) * GW)
        q4 = slice(b_idx * (D // 4), (b_idx + 1) * (D // 4))
        blob = np.zeros((IN_ROWS, S), np.float16)
        blob[RX:RX + GW] = x[b_idx][:, g * GW:(g + 1) * GW].T
        blob[RWQ:RWQ + 128] = (Wq[q4, cols] * 0.125).astype(np.float16).reshape(128, S)
        blob[RWK:RWK + 128] = Wk[q4, cols].astype(np.float16).reshape(128, S)
        blob[RWO:RWO + 128] = Wo[g * GW + b_idx * P:g * GW + (b_idx + 1) * P, :]
        blob[RWV:RWV + 16] = wv_sh[q4, :].astype(np.float16).reshape(16, S)
        blob[RB, 0:GW] = bq[cols] * 0.125
        blob[RB, GW:S] = bk[cols]
        blob[RBV, 0:DH] = bv_sh
        in_maps.append({"inb": blob})
    return in_maps, wv_sh, bv_sh


def kernel(x, Wq, bq, Wk, bk, Wv, bv, Wo, bo):
    x = np.asarray(x, dtype=np.float32)
    Wq = np.asarray(Wq, dtype=np.float32); bq = np.asarray(bq, dtype=np.float32)
    Wk = np.asarray(Wk, dtype=np.float32); bk = np.asarray(bk, dtype=np.float32)
    Wv = np.asarray(Wv, dtype=np.float32); bv = np.asarray(bv, dtype=np.float32)
    Wo = np.asarray(Wo, dtype=np.float32); bo = np.asarray(bo, dtype=np.float32)

    if "nc" not in _cached:
        _cached["nc"] = _build()
    nc = _cached["nc"]

    in_maps, wv_sh, bv_sh = _make_in_maps(x, Wq, bq, Wk, bk, Wv, bv, Wo, bo)
    res = run_bass_kernel_spmd(nc, in_maps, list(range(N_CORES)))
    r = res.results

    x_out = np.empty((B, S, D), dtype=np.float32)
    avg = np.empty((B, S, S), dtype=np.float32)
    for b_idx in range(B):
        x_out[b_idx] = r[2 * b_idx]["outb"][0:S, :].T.astype(np.float32) + bo
        avg[b_idx] = r[2 * b_idx + 1]["outb"][0:S, :].T.astype(np.float32)

    # ---- host fixup of rows with sparsemax support >= 8 ----
    flagged = []   # (b, head, i, tau_dev)
    for c in range(N_CORES):
        fl = r[c]["outb"][OFLAG:OFLAG + 8, :].reshape(P, HG * NT)
        taus = r[c]["outb"][OTAU:OTAU + 8, :].reshape(P, HG * NT)
        ps, gs = np.nonzero(fl > 0.5)
        for p, g64 in zip(ps, gs):
            head = (c % 2) * HG + g64 // NT
            i = (g64 % NT) * P + int(p)
            flagged.append((c // 2, head, i, float(taus[p, g64])))

    if flagged:
        bs_needed = sorted({f[0] for f in flagged})
        qkv_cache = {}
        for b_idx in bs_needed:
            qkv_cache[b_idx] = (
                x[b_idx] @ Wq + bq,
                x[b_idx] @ Wk + bk,
                x[b_idx] @ wv_sh + bv_sh,
            )
        scale = 1.0 / np.sqrt(DH)
        for b_idx, head, i, tau_dev in flagged:
            qb, kb, vb = qkv_cache[b_idx]
            hc = slice(head * DH, (head + 1) * DH)
            z = (qb[i, hc] @ kb[:, hc].T) * scale          # (S,)
            probs_new = _sparsemax_row(z)
            probs_old = np.maximum(z - tau_dev, 0.0)
            delta = probs_new - probs_old
            avg[b_idx, i, :] += delta / H
            x_out[b_idx, i, :] += (delta @ vb) @ Wo[hc, :]

    return x_out, avg
